# revision 4
# baseline (speedup 1.0000x reference)
"""CPChannelAttention Trainium2 kernel (8-core SPMD, two device phases).

Per patch n (of B*M=1024; FM_n = (C=256, P=49) slab of x):
  G_n   = FM_n @ FM_n.T                (fp32 gram on PE; FM^T staged via
                                        PE-transpose of patch-contiguous slabs)
  Sc_n  = softmax(G_n, axis=-1)        (ACT exp with per-partition -max bias +
                                        free-running sum; normalize via
                                        table-free ACT copy with 1/sum scale)
  cov_n = G_n/49 - mu mu^T             (rank-1 PSUM accumulate + scaled copy)
  Ec_n  = (Sc_n + cov_cp) @ FM_n       (lhsT = Sc^T + cov_cp^T; Sc^T comes from
                                        the same symmetric gram:
                                        Sc^T = exp(G + corr[q]) with corr
                                        injected as a K=2 bf16 hi/lo matmul)
corr[c] = -(max_c + ln sum_c) is produced in phase 1 (Ln batched per 32-patch
group to avoid ACT table thrash) and round-trips through DRAM; the tiny CP-ALS
on the (1,C,C) global covariance runs on host between phases.

Sharding: patch-row parallel — core k owns patch rows [4k, 4k+4) = 128 patches,
i.e. x rows [28k, 28k+28). All outputs shard the same way; no collectives.
"""
import numpy as np
from contextlib import ExitStack

import concourse.bass as bass
from concourse import bacc
import concourse.tile as tile
import concourse.mybir as mybir
from concourse import bass_utils

F32 = mybir.dt.float32
BF16 = mybir.dt.bfloat16
AF = mybir.ActivationFunctionType
AX = mybir.AxisListType
ALU = mybir.AluOpType

C = 256
H = W = 224
PH = PW = 7
HH = WW = 32          # patch grid
RANK, N_ITER = 8, 5
N_CORES = 8
ROWS_PER_CORE = HH // N_CORES      # 4 patch rows / core
NP_CORE = ROWS_PER_CORE * WW       # 128 patches / core
HS = ROWS_PER_CORE * PH            # 28 x-rows / core
DMA_BATCH = 8                      # patches per Sc/cov store DMA

_cache = {}


def _load_group(nc, sb_xg, sb_xp, xs_d, r):
    """Load one patch-row of x (both channel chunks) and stage a
    patch-contiguous copy xp (128, 32, 49) for matmul operands."""
    xg, xp = [], []
    for cc in range(2):
        xgc = sb_xg.tile([128, 7, 224], F32, tag=f"xg{cc}", name=f"xg{cc}")
        nc.sync.dma_start(
            xgc[:], xs_d[cc * 128:(cc + 1) * 128, r * 7:(r + 1) * 7, :])
        xpc = sb_xp.tile([128, WW, 49], F32, tag=f"xp{cc}", name=f"xp{cc}")
        nc.vector.tensor_copy(
            xpc[:], xgc[:].rearrange("c h (n w) -> c n h w", n=WW))
        xg.append(xgc)
        xp.append(xpc)
    return xg, xp


def _gram_ops(nc, ps_tp, ps_gs, sb_fmt, xp, ww, iden):
    """Per-patch: transpose both channel chunks to FMT = FM^T, fp32 gram.
    Returns (fmt, gs); gs = (128, 2, 256) PSUM holding the raw gram G."""
    tp = ps_tp.tile([49, 256], F32, tag="tp", name="tp")
    nc.tensor.matmul(tp[:, 0:128], xp[0][:, ww, :], iden[:],
                     is_transpose=True, start=True, stop=True)
    nc.tensor.matmul(tp[:, 128:256], xp[1][:, ww, :], iden[:],
                     is_transpose=True, start=False, stop=True,
                     skip_group_check=True)
    fmt = sb_fmt.tile([49, 256], F32, tag="fmt", name="fmt")
    nc.any.tensor_copy(fmt[:], tp[:])
    gs = ps_gs.tile([128, 2, 256], F32, tag="gs", name="gs")
    nc.tensor.matmul(gs[:, 0, :], fmt[:, 0:128], fmt[:], start=True, stop=True)
    nc.tensor.matmul(gs[:, 1, :], fmt[:, 128:256], fmt[:], start=False,
                     stop=True, skip_group_check=True)
    return fmt, gs


def _build_phase1():
    nc = bacc.Bacc("TRN2", target_bir_lowering=False, debug=False)
    xs_d = nc.dram_tensor("xs", [C, HS, W], F32, kind="ExternalInput").ap()
    iden_d = nc.dram_tensor("iden", [128, 128], F32, kind="ExternalInput").ap()
    sc_d = nc.dram_tensor("sc", [NP_CORE, C, C], F32, kind="ExternalOutput").ap()
    cov_d = nc.dram_tensor("cov", [NP_CORE, C, C], F32, kind="ExternalOutput").ap()
    corr_d = nc.dram_tensor("corr", [ROWS_PER_CORE, 64, 128], F32,
                            kind="ExternalOutput").ap()

    with tile.TileContext(nc) as tc, ExitStack() as ctx:
        sb_const = ctx.enter_context(tc.tile_pool(name="const", bufs=1))
        sb_xg = ctx.enter_context(tc.tile_pool(name="xg", bufs=2))
        sb_xp = ctx.enter_context(tc.tile_pool(name="xp", bufs=2))
        sb_fmt = ctx.enter_context(tc.tile_pool(name="fmt", bufs=3))
        sb_e = ctx.enter_context(tc.tile_pool(name="e", bufs=3))
        sb_small = ctx.enter_context(tc.tile_pool(name="small", bufs=4))
        sb_stage = ctx.enter_context(tc.tile_pool(name="stage", bufs=2))
        sb_grp = ctx.enter_context(tc.tile_pool(name="grp", bufs=2))
        ps_tp = ctx.enter_context(tc.tile_pool(name="pstp", bufs=2, space="PSUM"))
        ps_gs = ctx.enter_context(tc.tile_pool(name="psgs", bufs=2, space="PSUM"))
        ps_mu = ctx.enter_context(tc.tile_pool(name="psmu", bufs=2, space="PSUM"))
        ps_ct = ctx.enter_context(tc.tile_pool(name="psct", bufs=1, space="PSUM"))

        iden = sb_const.tile([128, 128], F32)
        nc.sync.dma_start(iden[:], iden_d[:])
        ones49 = sb_const.tile([49, 1], F32)
        nc.vector.memset(ones49[:], 1.0)

        for r in range(ROWS_PER_CORE):
            xg, xp = _load_group(nc, sb_xg, sb_xp, xs_d, r)
            mxn_g = sb_grp.tile([128, 64], F32, tag="mxn_g", name="mxn_g")
            ssum_g = sb_grp.tile([128, 64], F32, tag="ssum_g", name="ssum_g")
            for b in range(WW // DMA_BATCH):
                sc_stage = sb_stage.tile([128, DMA_BATCH, 2, 256], F32,
                                         tag="scs", name="sc_stage")
                cov_stage = sb_stage.tile([128, DMA_BATCH, 2, 256], F32,
                                          tag="covs", name="cov_stage")
                for i in range(DMA_BATCH):
                    ww = b * DMA_BATCH + i
                    fmt, gs = _gram_ops(nc, ps_tp, ps_gs, sb_fmt, xp, ww, iden)
                    # column sums -> s = 49*mu (1, 256)
                    mus = ps_mu.tile([1, 256], F32, tag="mus", name="mus")
                    nc.tensor.matmul(mus[:], ones49[:], fmt[:], start=True,
                                     stop=True)
                    # softmax: -max, exp(G - max) with running sum
                    nc.vector.reduce_max(mxn_g[:, 2 * ww:2 * ww + 2], gs[:],
                                         axis=AX.X, negate=True)
                    e_sb = sb_e.tile([128, 2, 256], F32, tag="e", name="e_sb")
                    for a in range(2):
                        nc.scalar.activation(
                            e_sb[:, a, :], gs[:, a, :], AF.Exp,
                            bias=mxn_g[:, 2 * ww + a:2 * ww + a + 1],
                            accum_out=ssum_g[:, 2 * ww + a:2 * ww + a + 1])
                    rsum = sb_small.tile([128, 2], F32, tag="rsum", name="rsum")
                    nc.vector.reciprocal(rsum[:], ssum_g[:, 2 * ww:2 * ww + 2])
                    # Sc = E * (1/sum)  (table-free ACT copy with scale AP)
                    for a in range(2):
                        nc.scalar.mul(sc_stage[:, i, a, :], e_sb[:, a, :],
                                      rsum[:, a:a + 1])
                    # cov = (G - s s^T/49)/49 via rank-1 accumulate
                    s_sb = sb_small.tile([1, 256], F32, tag="ssb", name="s_sb")
                    s_neg = sb_small.tile([1, 256], F32, tag="sneg", name="s_neg")
                    nc.vector.tensor_copy(s_sb[:], mus[:])
                    nc.vector.tensor_scalar_mul(s_neg[:], mus[:], -1.0 / 49.0)
                    nc.tensor.matmul(gs[:, 0, :], s_sb[0:1, 0:128], s_neg[:],
                                     start=False, stop=True,
                                     skip_group_check=True)
                    nc.tensor.matmul(gs[:, 1, :], s_sb[0:1, 128:256], s_neg[:],
                                     start=False, stop=True,
                                     skip_group_check=True)
                    nc.any.tensor_scalar_mul(cov_stage[:, i, :, :], gs[:],
                                             1.0 / 49.0)
                n0 = r * WW + b * DMA_BATCH
                nc.sync.dma_start(
                    sc_d[n0:n0 + DMA_BATCH].rearrange("n (a p) q -> p n a q", a=2),
                    sc_stage[:])
                nc.sync.dma_start(
                    cov_d[n0:n0 + DMA_BATCH].rearrange("n (a p) q -> p n a q", a=2),
                    cov_stage[:])
            # group-batched corr = -(max + ln sum), then transpose to rows
            lnsum_g = sb_grp.tile([128, 64], F32, tag="lnsum_g", name="lnsum_g")
            nc.scalar.activation(lnsum_g[:], ssum_g[:], AF.Ln)
            corr_gt = sb_grp.tile([128, 64], F32, tag="corr_gt", name="corr_gt")
            nc.vector.tensor_sub(corr_gt[:], mxn_g[:], lnsum_g[:])
            ct = ps_ct.tile([64, 128], F32, tag="ct", name="ct")
            nc.tensor.matmul(ct[:], corr_gt[:], iden[:], is_transpose=True,
                             start=True, stop=True)
            ct_sb = sb_grp.tile([64, 128], F32, tag="ctsb", name="ct_sb")
            nc.vector.tensor_copy(ct_sb[:], ct[:])
            nc.sync.dma_start(corr_d[r], ct_sb[:])
    nc.compile()
    return nc


def _build_phase2():
    nc = bacc.Bacc("TRN2", target_bir_lowering=False, debug=False)
    xs_d = nc.dram_tensor("xs", [C, HS, W], F32, kind="ExternalInput").ap()
    iden_d = nc.dram_tensor("iden", [128, 128], F32, kind="ExternalInput").ap()
    corr_d = nc.dram_tensor("corr", [ROWS_PER_CORE, 2, 64, 128], BF16,
                            kind="ExternalInput").ap()
    covt_d = nc.dram_tensor("covt", [128, 2, 256], F32, kind="ExternalInput").ap()
    beta_d = nc.dram_tensor("beta", [1, 1], F32, kind="ExternalInput").ap()
    ec_d = nc.dram_tensor("ec", [C, HS, W], F32, kind="ExternalOutput").ap()
    out_d = nc.dram_tensor("out", [C, HS, W], F32, kind="ExternalOutput").ap()

    with tile.TileContext(nc) as tc, ExitStack() as ctx:
        sb_const = ctx.enter_context(tc.tile_pool(name="const", bufs=1))
        sb_xg = ctx.enter_context(tc.tile_pool(name="xg", bufs=2))
        sb_xp = ctx.enter_context(tc.tile_pool(name="xp", bufs=2))
        sb_fmt = ctx.enter_context(tc.tile_pool(name="fmt", bufs=3))
        sb_lt = ctx.enter_context(tc.tile_pool(name="lt", bufs=3))
        sb_corr = ctx.enter_context(tc.tile_pool(name="corr", bufs=2))
        sb_stage = ctx.enter_context(tc.tile_pool(name="stage", bufs=2))
        ps_tp = ctx.enter_context(tc.tile_pool(name="pstp", bufs=2, space="PSUM"))
        ps_gs = ctx.enter_context(tc.tile_pool(name="psgs", bufs=2, space="PSUM"))
        ps_ec = ctx.enter_context(tc.tile_pool(name="psec", bufs=2, space="PSUM"))

        iden = sb_const.tile([128, 128], F32)
        nc.sync.dma_start(iden[:], iden_d[:])
        ones2 = sb_const.tile([2, 128], BF16)
        nc.vector.memset(ones2[:], 1.0)
        covt = sb_const.tile([128, 2, 256], F32)
        nc.sync.dma_start(covt[:], covt_d[:])
        beta_t = sb_const.tile([1, 1], F32)
        nc.sync.dma_start(beta_t[:], beta_d[:])
        beta_b = sb_const.tile([128, 1], F32)
        nc.gpsimd.partition_broadcast(beta_b[:], beta_t[:])

        for r in range(ROWS_PER_CORE):
            xg, xp = _load_group(nc, sb_xg, sb_xp, xs_d, r)
            corr_sb = sb_corr.tile([2, WW * 256], BF16, tag="corr_sb",
                                   name="corr_sb")
            nc.sync.dma_start(corr_sb[:], corr_d[r])
            ec_stage = [sb_stage.tile([128, 7, 224], F32, tag=f"ec{m}",
                                      name=f"ec_stage{m}")
                        for m in range(2)]
            for ww in range(WW):
                fmt, gs = _gram_ops(nc, ps_tp, ps_gs, sb_fmt, xp, ww, iden)
                # inject corr[q] into both chunks (bf16 hi/lo pair, K=2)
                for a in range(2):
                    nc.tensor.matmul(
                        gs[:, a, :], ones2[:],
                        corr_sb[:, ww * 256:(ww + 1) * 256],
                        start=False, stop=True, skip_group_check=True)
                # Sc^T = exp(G + corr); L^T = Sc^T + cov_cp^T
                sct = sb_lt.tile([128, 2, 256], F32, tag="sct", name="sct")
                nc.scalar.activation(sct[:], gs[:], AF.Exp)
                lt = sb_lt.tile([128, 2, 256], F32, tag="lt", name="lt")
                nc.gpsimd.tensor_add(lt[:], sct[:], covt[:])
                # Ec chunks: (128c, 49) = sum_a LT[:,a,m-cols]^T @ FM_a
                ec_ps = ps_ec.tile([128, 2, 49], F32, tag="ecps", name="ec_ps")
                first = True
                for m in range(2):
                    for a in range(2):
                        nc.tensor.matmul(
                            ec_ps[:, m, :], lt[:, a, m * 128:(m + 1) * 128],
                            xp[a][:, ww, :],
                            start=first, stop=True,
                            skip_group_check=not first)
                        first = False
                for m in range(2):
                    nc.any.tensor_copy(
                        ec_stage[m][:, :, ww * 7:(ww + 1) * 7],
                        ec_ps[:, m, :].rearrange("c (h w) -> c h w", h=7))
            for m in range(2):
                nc.sync.dma_start(
                    ec_d[m * 128:(m + 1) * 128, r * 7:(r + 1) * 7, :],
                    ec_stage[m][:])
                att = sb_stage.tile([128, 7, 224], F32, tag=f"att{m}",
                                    name=f"att{m}")
                nc.vector.scalar_tensor_tensor(
                    att[:], ec_stage[m][:], beta_b[:, 0:1], xg[m][:],
                    op0=ALU.mult, op1=ALU.add)
                outm = sb_stage.tile([128, 7, 224], F32, tag=f"out{m}",
                                     name=f"outm{m}")
                nc.vector.tensor_mul(outm[:], att[:], xg[m][:])
                nc.sync.dma_start(
                    out_d[m * 128:(m + 1) * 128, r * 7:(r + 1) * 7, :], outm[:])
    nc.compile()
    return nc


def _kr(A, B):
    return (A[:, None, :] * B[None, :, :]).reshape(-1, A.shape[1])


def _unfold_mode(T, mode):
    return np.moveaxis(T, mode, 0).reshape(T.shape[mode], -1)


def _parafac_cov_cp(cov_global, f0, f1, f2):
    """Mirror of the reference CP-ALS (float32), returns cov_cp (C, C)."""
    T = cov_global.reshape(1, C, C).astype(np.float32)
    factors = [f0.astype(np.float32), f1.astype(np.float32),
               f2.astype(np.float32)]
    for _ in range(N_ITER):
        for mode in range(3):
            others = [f for i, f in enumerate(factors) if i != mode]
            kr = _kr(others[0], others[1])
            mttkrp = _unfold_mode(T, mode) @ kr
            V = (others[0].T @ others[0]) * (others[1].T @ others[1])
            factors[mode] = np.linalg.solve(V.T, mttkrp.T).T
    w = factors[0][0]
    return (factors[1] * w[None, :]) @ factors[2].T


def kernel(x, beta, f0_init, f1_init, f2_init):
    import ml_dtypes
    x = np.asarray(x, dtype=np.float32)
    beta = np.asarray(beta, dtype=np.float32)
    iden = np.eye(128, dtype=np.float32)

    if "p1" not in _cache:
        _cache["p1"] = _build_phase1()
    nc1 = _cache["p1"]
    in_maps1 = []
    for k in range(N_CORES):
        xs = np.ascontiguousarray(x[0, :, k * HS:(k + 1) * HS, :])
        in_maps1.append({"xs": xs, "iden": iden})
    res1 = bass_utils.run_bass_kernel_spmd(nc1, in_maps1,
                                           core_ids=list(range(N_CORES)))

    sc_full = np.empty((N_CORES * NP_CORE, C, C), np.float32)
    cov_full = np.empty((N_CORES * NP_CORE, C, C), np.float32)
    for k in range(N_CORES):
        sc_full[k * NP_CORE:(k + 1) * NP_CORE] = res1.results[k]["sc"]
        cov_full[k * NP_CORE:(k + 1) * NP_CORE] = res1.results[k]["cov"]

    cov_global = cov_full.mean(axis=0, dtype=np.float32)
    cov_cp = _parafac_cov_cp(cov_global, f0_init, f1_init, f2_init)
    covt = np.ascontiguousarray(
        cov_cp.T.reshape(2, 128, 256).transpose(1, 0, 2)).astype(np.float32)

    if "p2" not in _cache:
        _cache["p2"] = _build_phase2()
    nc2 = _cache["p2"]
    in_maps2 = []
    for k in range(N_CORES):
        corr_f = res1.results[k]["corr"].astype(np.float32)      # (R, 64, 128)
        hi = corr_f.astype(ml_dtypes.bfloat16)
        lo = (corr_f - hi.astype(np.float32)).astype(ml_dtypes.bfloat16)
        corr2 = np.stack([hi, lo], axis=1)                       # (R, 2, 64, 128)
        in_maps2.append({
            "xs": in_maps1[k]["xs"],
            "iden": iden,
            "corr": np.ascontiguousarray(corr2),
            "covt": covt,
            "beta": beta.reshape(1, 1),
        })
    res2 = bass_utils.run_bass_kernel_spmd(nc2, in_maps2,
                                           core_ids=list(range(N_CORES)))

    ec_map = np.empty((1, C, H, W), np.float32)
    out = np.empty((1, C, H, W), np.float32)
    for k in range(N_CORES):
        ec_map[0, :, k * HS:(k + 1) * HS, :] = res2.results[k]["ec"]
        out[0, :, k * HS:(k + 1) * HS, :] = res2.results[k]["out"]

    return (out,
            sc_full.reshape(1, N_CORES * NP_CORE, C, C),
            cov_full.reshape(1, N_CORES * NP_CORE, C, C),
            ec_map)


# revision 5
# speedup vs baseline: 1.2622x; 1.2622x over previous
"""CPChannelAttention Trainium2 kernel (8-core SPMD, two device phases).

Per patch n (of B*M=1024; FM_n = (C=256, P=49) slab of x):
  G_n   = FM_n @ FM_n.T                (fp32 gram on PE; FM^T staged via
                                        PE-transpose of patch-contiguous slabs)
  Sc_n  = softmax(G_n, axis=-1)        (ACT exp with per-partition -max bias +
                                        free-running sum; normalize via
                                        table-free ACT copy with 1/sum scale)
  cov_n = G_n/49 - mu mu^T             (rank-1 PSUM accumulate + scaled copy)
  Ec_n  = (Sc_n + cov_cp) @ FM_n       (lhsT = Sc^T + cov_cp^T; Sc^T comes from
                                        the same symmetric gram:
                                        Sc^T = exp(G + corr[q]) with corr
                                        injected as a K=2 bf16 hi/lo matmul)
corr[c] = -(max_c + ln sum_c) is produced in phase 1 (Ln batched per 32-patch
group to avoid ACT table thrash) and round-trips through DRAM; the tiny CP-ALS
on the (1,C,C) global covariance runs on host between phases.

Sharding: patch-row parallel — core k owns patch rows [4k, 4k+4) = 128 patches,
i.e. x rows [28k, 28k+28). All outputs shard the same way; no collectives.
"""
import numpy as np
from contextlib import ExitStack

import concourse.bass as bass
from concourse import bacc
import concourse.tile as tile
import concourse.mybir as mybir
from concourse import bass_utils

F32 = mybir.dt.float32
BF16 = mybir.dt.bfloat16
AF = mybir.ActivationFunctionType
AX = mybir.AxisListType
ALU = mybir.AluOpType

C = 256
H = W = 224
PH = PW = 7
HH = WW = 32          # patch grid
RANK, N_ITER = 8, 5
N_CORES = 8
ROWS_PER_CORE = HH // N_CORES      # 4 patch rows / core
NP_CORE = ROWS_PER_CORE * WW       # 128 patches / core
HS = ROWS_PER_CORE * PH            # 28 x-rows / core
DMA_BATCH = 8                      # patches per Sc/cov store DMA

_cache = {}


def _load_group(nc, sb_xg, sb_xp, xs_d, r):
    """Load one patch-row of x (both channel chunks) and stage a
    patch-contiguous copy xp (128, 32, 49) for matmul operands."""
    xg, xp = [], []
    for cc in range(2):
        xgc = sb_xg.tile([128, 7, 224], F32, tag=f"xg{cc}", name=f"xg{cc}")
        nc.sync.dma_start(
            xgc[:], xs_d[cc * 128:(cc + 1) * 128, r * 7:(r + 1) * 7, :])
        xpc = sb_xp.tile([128, WW, 49], F32, tag=f"xp{cc}", name=f"xp{cc}")
        nc.vector.tensor_copy(
            xpc[:], xgc[:].rearrange("c h (n w) -> c n h w", n=WW))
        xg.append(xgc)
        xp.append(xpc)
    return xg, xp


def _gram_ops(nc, ps_tp, ps_gs, sb_fmt, xp, ww, iden):
    """Per-patch: transpose both channel chunks to FMT = FM^T, fp32 gram.
    Returns (fmt, gs); gs = (128, 2, 256) PSUM holding the raw gram G."""
    tp = ps_tp.tile([49, 256], F32, tag="tp", name="tp")
    nc.tensor.matmul(tp[:, 0:128], xp[0][:, ww, :], iden[:],
                     is_transpose=True, start=True, stop=True)
    nc.tensor.matmul(tp[:, 128:256], xp[1][:, ww, :], iden[:],
                     is_transpose=True, start=False, stop=True,
                     skip_group_check=True)
    fmt = sb_fmt.tile([49, 256], F32, tag="fmt", name="fmt")
    nc.any.tensor_copy(fmt[:], tp[:])
    gs = ps_gs.tile([128, 2, 256], F32, tag="gs", name="gs")
    nc.tensor.matmul(gs[:, 0, :], fmt[:, 0:128], fmt[:], start=True, stop=True)
    nc.tensor.matmul(gs[:, 1, :], fmt[:, 128:256], fmt[:], start=False,
                     stop=True, skip_group_check=True)
    return fmt, gs


def _build_phase1():
    nc = bacc.Bacc("TRN2", target_bir_lowering=False, debug=False)
    xs_d = nc.dram_tensor("xs", [C, HS, W], F32, kind="ExternalInput").ap()
    iden_d = nc.dram_tensor("iden", [128, 128], F32, kind="ExternalInput").ap()
    sc_d = nc.dram_tensor("sc", [NP_CORE, C, C], F32, kind="ExternalOutput").ap()
    cov_d = nc.dram_tensor("cov", [NP_CORE, C, C], F32, kind="ExternalOutput").ap()
    corr_d = nc.dram_tensor("corr", [ROWS_PER_CORE, 64, 128], F32,
                            kind="ExternalOutput").ap()

    with tile.TileContext(nc) as tc, ExitStack() as ctx:
        sb_const = ctx.enter_context(tc.tile_pool(name="const", bufs=1))
        sb_xg = ctx.enter_context(tc.tile_pool(name="xg", bufs=2))
        sb_xp = ctx.enter_context(tc.tile_pool(name="xp", bufs=2))
        sb_fmt = ctx.enter_context(tc.tile_pool(name="fmt", bufs=3))
        sb_e = ctx.enter_context(tc.tile_pool(name="e", bufs=3))
        sb_small = ctx.enter_context(tc.tile_pool(name="small", bufs=4))
        sb_stage = ctx.enter_context(tc.tile_pool(name="stage", bufs=2))
        sb_grp = ctx.enter_context(tc.tile_pool(name="grp", bufs=2))
        ps_tp = ctx.enter_context(tc.tile_pool(name="pstp", bufs=2, space="PSUM"))
        ps_gs = ctx.enter_context(tc.tile_pool(name="psgs", bufs=4, space="PSUM"))
        ps_mu = ctx.enter_context(tc.tile_pool(name="psmu", bufs=1, space="PSUM"))
        ps_ct = ctx.enter_context(tc.tile_pool(name="psct", bufs=1, space="PSUM"))

        iden = sb_const.tile([128, 128], F32)
        nc.sync.dma_start(iden[:], iden_d[:])
        ones49 = sb_const.tile([49, 1], F32)
        nc.vector.memset(ones49[:], 1.0)

        for r in range(ROWS_PER_CORE):
            xg, xp = _load_group(nc, sb_xg, sb_xp, xs_d, r)
            mxn_g = sb_grp.tile([128, 64], F32, tag="mxn_g", name="mxn_g")
            ssum_g = sb_grp.tile([128, 64], F32, tag="ssum_g", name="ssum_g")
            for b in range(WW // DMA_BATCH):
                sc_stage = sb_stage.tile([128, DMA_BATCH, 2, 256], F32,
                                         tag="scs", name="sc_stage")
                cov_stage = sb_stage.tile([128, DMA_BATCH, 2, 256], F32,
                                          tag="covs", name="cov_stage")
                for i in range(DMA_BATCH):
                    ww = b * DMA_BATCH + i
                    fmt, gs = _gram_ops(nc, ps_tp, ps_gs, sb_fmt, xp, ww, iden)
                    # column sums -> s = 49*mu (1, 256)
                    mus = ps_mu.tile([1, 256], F32, tag="mus", name="mus")
                    nc.tensor.matmul(mus[:], ones49[:], fmt[:], start=True,
                                     stop=True)
                    # softmax: -max, exp(G - max) with running sum
                    nc.vector.reduce_max(mxn_g[:, 2 * ww:2 * ww + 2], gs[:],
                                         axis=AX.X, negate=True)
                    e_sb = sb_e.tile([128, 2, 256], F32, tag="e", name="e_sb")
                    for a in range(2):
                        nc.scalar.activation(
                            e_sb[:, a, :], gs[:, a, :], AF.Exp,
                            bias=mxn_g[:, 2 * ww + a:2 * ww + a + 1],
                            accum_out=ssum_g[:, 2 * ww + a:2 * ww + a + 1])
                    rsum = sb_small.tile([128, 2], F32, tag="rsum", name="rsum")
                    nc.vector.reciprocal(rsum[:], ssum_g[:, 2 * ww:2 * ww + 2])
                    # Sc = E * (1/sum)  (table-free ACT copy with scale AP)
                    for a in range(2):
                        nc.scalar.mul(sc_stage[:, i, a, :], e_sb[:, a, :],
                                      rsum[:, a:a + 1])
                    # cov = (G - s s^T/49)/49 via bf16 rank-1 accumulate
                    s_sb = sb_small.tile([1, 256], BF16, tag="ssb", name="s_sb")
                    s_neg = sb_small.tile([1, 256], BF16, tag="sneg", name="s_neg")
                    nc.vector.tensor_copy(s_sb[:], mus[:])
                    nc.vector.tensor_scalar_mul(s_neg[:], mus[:], -1.0 / 49.0)
                    nc.tensor.matmul(gs[:, 0, :], s_sb[0:1, 0:128], s_neg[:],
                                     start=False, stop=True,
                                     skip_group_check=True)
                    nc.tensor.matmul(gs[:, 1, :], s_sb[0:1, 128:256], s_neg[:],
                                     start=False, stop=True,
                                     skip_group_check=True)
                    nc.any.tensor_scalar_mul(cov_stage[:, i, :, :], gs[:],
                                             1.0 / 49.0)
                n0 = r * WW + b * DMA_BATCH
                nc.sync.dma_start(
                    sc_d[n0:n0 + DMA_BATCH].rearrange("n (a p) q -> p n a q", a=2),
                    sc_stage[:])
                nc.sync.dma_start(
                    cov_d[n0:n0 + DMA_BATCH].rearrange("n (a p) q -> p n a q", a=2),
                    cov_stage[:])
            # group-batched corr = -(max + ln sum), then transpose to rows
            lnsum_g = sb_grp.tile([128, 64], F32, tag="lnsum_g", name="lnsum_g")
            nc.scalar.activation(lnsum_g[:], ssum_g[:], AF.Ln)
            corr_gt = sb_grp.tile([128, 64], F32, tag="corr_gt", name="corr_gt")
            nc.vector.tensor_sub(corr_gt[:], mxn_g[:], lnsum_g[:])
            ct = ps_ct.tile([64, 128], F32, tag="ct", name="ct")
            nc.tensor.matmul(ct[:], corr_gt[:], iden[:], is_transpose=True,
                             start=True, stop=True)
            ct_sb = sb_grp.tile([64, 128], F32, tag="ctsb", name="ct_sb")
            nc.vector.tensor_copy(ct_sb[:], ct[:])
            nc.sync.dma_start(corr_d[r], ct_sb[:])
    nc.compile()
    return nc


def _build_phase2():
    nc = bacc.Bacc("TRN2", target_bir_lowering=False, debug=False)
    xs_d = nc.dram_tensor("xs", [C, HS, W], F32, kind="ExternalInput").ap()
    iden_d = nc.dram_tensor("iden", [128, 128], F32, kind="ExternalInput").ap()
    corr_d = nc.dram_tensor("corr", [ROWS_PER_CORE, 2, 64, 128], BF16,
                            kind="ExternalInput").ap()
    covt_d = nc.dram_tensor("covt", [128, 2, 256], F32, kind="ExternalInput").ap()
    beta_d = nc.dram_tensor("beta", [1, 1], F32, kind="ExternalInput").ap()
    ec_d = nc.dram_tensor("ec", [C, HS, W], F32, kind="ExternalOutput").ap()
    out_d = nc.dram_tensor("out", [C, HS, W], F32, kind="ExternalOutput").ap()

    with tile.TileContext(nc) as tc, ExitStack() as ctx:
        sb_const = ctx.enter_context(tc.tile_pool(name="const", bufs=1))
        sb_xg = ctx.enter_context(tc.tile_pool(name="xg", bufs=2))
        sb_xp = ctx.enter_context(tc.tile_pool(name="xp", bufs=2))
        sb_fmt = ctx.enter_context(tc.tile_pool(name="fmt", bufs=3))
        sb_lt = ctx.enter_context(tc.tile_pool(name="lt", bufs=3))
        sb_corr = ctx.enter_context(tc.tile_pool(name="corr", bufs=2))
        sb_stage = ctx.enter_context(tc.tile_pool(name="stage", bufs=2))
        ps_tp = ctx.enter_context(tc.tile_pool(name="pstp", bufs=2, space="PSUM"))
        ps_gs = ctx.enter_context(tc.tile_pool(name="psgs", bufs=4, space="PSUM"))
        ps_ec = ctx.enter_context(tc.tile_pool(name="psec", bufs=2, space="PSUM"))

        iden = sb_const.tile([128, 128], F32)
        nc.sync.dma_start(iden[:], iden_d[:])
        ones2 = sb_const.tile([2, 128], BF16)
        nc.vector.memset(ones2[:], 1.0)
        covt = sb_const.tile([128, 2, 256], F32)
        nc.sync.dma_start(covt[:], covt_d[:])
        beta_t = sb_const.tile([1, 1], F32)
        nc.sync.dma_start(beta_t[:], beta_d[:])
        beta_b = sb_const.tile([128, 1], F32)
        nc.gpsimd.partition_broadcast(beta_b[:], beta_t[:])

        for r in range(ROWS_PER_CORE):
            xg, xp = _load_group(nc, sb_xg, sb_xp, xs_d, r)
            xpb = []
            for cc in range(2):
                xpbc = sb_xp.tile([128, WW, 49], BF16, tag=f"xpb{cc}",
                                  name=f"xpb{cc}")
                nc.vector.tensor_copy(xpbc[:], xp[cc][:])
                xpb.append(xpbc)
            corr_sb = sb_corr.tile([2, WW * 256], BF16, tag="corr_sb",
                                   name="corr_sb")
            nc.sync.dma_start(corr_sb[:], corr_d[r])
            ec_stage = [sb_stage.tile([128, 7, 224], F32, tag=f"ec{m}",
                                      name=f"ec_stage{m}")
                        for m in range(2)]
            for ww in range(WW):
                fmt, gs = _gram_ops(nc, ps_tp, ps_gs, sb_fmt, xp, ww, iden)
                # inject corr[q] into both chunks (bf16 hi/lo pair, K=2)
                for a in range(2):
                    nc.tensor.matmul(
                        gs[:, a, :], ones2[:],
                        corr_sb[:, ww * 256:(ww + 1) * 256],
                        start=False, stop=True, skip_group_check=True)
                # Sc^T = exp(G + corr); L^T = Sc^T + cov_cp^T (bf16 for Ec)
                sct = sb_lt.tile([128, 2, 256], F32, tag="sct", name="sct")
                nc.scalar.activation(sct[:], gs[:], AF.Exp)
                lt = sb_lt.tile([128, 2, 256], BF16, tag="lt", name="lt")
                nc.gpsimd.tensor_add(lt[:], sct[:], covt[:])
                # Ec chunks: (128c, 49) = sum_a LT[:,a,m-cols]^T @ FM_a
                ec_ps = ps_ec.tile([128, 2, 49], F32, tag="ecps", name="ec_ps")
                first = True
                for m in range(2):
                    for a in range(2):
                        nc.tensor.matmul(
                            ec_ps[:, m, :], lt[:, a, m * 128:(m + 1) * 128],
                            xpb[a][:, ww, :],
                            start=first, stop=True,
                            skip_group_check=not first)
                        first = False
                for m in range(2):
                    nc.any.tensor_copy(
                        ec_stage[m][:, :, ww * 7:(ww + 1) * 7],
                        ec_ps[:, m, :].rearrange("c (h w) -> c h w", h=7))
            for m in range(2):
                nc.sync.dma_start(
                    ec_d[m * 128:(m + 1) * 128, r * 7:(r + 1) * 7, :],
                    ec_stage[m][:])
                att = sb_stage.tile([128, 7, 224], F32, tag=f"att{m}",
                                    name=f"att{m}")
                nc.vector.scalar_tensor_tensor(
                    att[:], ec_stage[m][:], beta_b[:, 0:1], xg[m][:],
                    op0=ALU.mult, op1=ALU.add)
                outm = sb_stage.tile([128, 7, 224], F32, tag=f"out{m}",
                                     name=f"outm{m}")
                nc.vector.tensor_mul(outm[:], att[:], xg[m][:])
                nc.sync.dma_start(
                    out_d[m * 128:(m + 1) * 128, r * 7:(r + 1) * 7, :], outm[:])
    nc.compile()
    return nc


def _kr(A, B):
    return (A[:, None, :] * B[None, :, :]).reshape(-1, A.shape[1])


def _unfold_mode(T, mode):
    return np.moveaxis(T, mode, 0).reshape(T.shape[mode], -1)


def _parafac_cov_cp(cov_global, f0, f1, f2):
    """Mirror of the reference CP-ALS (float32), returns cov_cp (C, C)."""
    T = cov_global.reshape(1, C, C).astype(np.float32)
    factors = [f0.astype(np.float32), f1.astype(np.float32),
               f2.astype(np.float32)]
    for _ in range(N_ITER):
        for mode in range(3):
            others = [f for i, f in enumerate(factors) if i != mode]
            kr = _kr(others[0], others[1])
            mttkrp = _unfold_mode(T, mode) @ kr
            V = (others[0].T @ others[0]) * (others[1].T @ others[1])
            factors[mode] = np.linalg.solve(V.T, mttkrp.T).T
    w = factors[0][0]
    return (factors[1] * w[None, :]) @ factors[2].T


def kernel(x, beta, f0_init, f1_init, f2_init):
    import ml_dtypes
    x = np.asarray(x, dtype=np.float32)
    beta = np.asarray(beta, dtype=np.float32)
    iden = np.eye(128, dtype=np.float32)

    if "p1" not in _cache:
        _cache["p1"] = _build_phase1()
    nc1 = _cache["p1"]
    in_maps1 = []
    for k in range(N_CORES):
        xs = np.ascontiguousarray(x[0, :, k * HS:(k + 1) * HS, :])
        in_maps1.append({"xs": xs, "iden": iden})
    res1 = bass_utils.run_bass_kernel_spmd(nc1, in_maps1,
                                           core_ids=list(range(N_CORES)))

    sc_full = np.empty((N_CORES * NP_CORE, C, C), np.float32)
    cov_full = np.empty((N_CORES * NP_CORE, C, C), np.float32)
    for k in range(N_CORES):
        sc_full[k * NP_CORE:(k + 1) * NP_CORE] = res1.results[k]["sc"]
        cov_full[k * NP_CORE:(k + 1) * NP_CORE] = res1.results[k]["cov"]

    cov_global = cov_full.mean(axis=0, dtype=np.float32)
    cov_cp = _parafac_cov_cp(cov_global, f0_init, f1_init, f2_init)
    covt = np.ascontiguousarray(
        cov_cp.T.reshape(2, 128, 256).transpose(1, 0, 2)).astype(np.float32)

    if "p2" not in _cache:
        _cache["p2"] = _build_phase2()
    nc2 = _cache["p2"]
    in_maps2 = []
    for k in range(N_CORES):
        corr_f = res1.results[k]["corr"].astype(np.float32)      # (R, 64, 128)
        hi = corr_f.astype(ml_dtypes.bfloat16)
        lo = (corr_f - hi.astype(np.float32)).astype(ml_dtypes.bfloat16)
        corr2 = np.stack([hi, lo], axis=1)                       # (R, 2, 64, 128)
        in_maps2.append({
            "xs": in_maps1[k]["xs"],
            "iden": iden,
            "corr": np.ascontiguousarray(corr2),
            "covt": covt,
            "beta": beta.reshape(1, 1),
        })
    res2 = bass_utils.run_bass_kernel_spmd(nc2, in_maps2,
                                           core_ids=list(range(N_CORES)))

    ec_map = np.empty((1, C, H, W), np.float32)
    out = np.empty((1, C, H, W), np.float32)
    for k in range(N_CORES):
        ec_map[0, :, k * HS:(k + 1) * HS, :] = res2.results[k]["ec"]
        out[0, :, k * HS:(k + 1) * HS, :] = res2.results[k]["out"]

    return (out,
            sc_full.reshape(1, N_CORES * NP_CORE, C, C),
            cov_full.reshape(1, N_CORES * NP_CORE, C, C),
            ec_map)
